# revision 19
# baseline (speedup 1.0000x reference)
"""Causal self-attention (RoPE + qk-RMS-norm) Trainium2 kernel.

Sharding: 8 cores = 2 batches x 4 head-groups (tensor-parallel over heads,
data-parallel over batch). Each core computes its head-group's attention and
a row-parallel partial of the output projection; the host sums the 4
per-group partials per batch (the all-reduce of row-parallel sharding).

Per-core layout: Q.T/K.T computed directly in [d, t] (no transposes),
V in [t, d]. Transposed flash attention: S.T = K @ Q.T so P.T feeds the
PV matmul directly; softmax has no max-subtraction (RMS-normed scores are
bounded by sqrt(D)); column sums via ones-matmul; 1/sum deferred to Y.T.
Matmuls run in float32r (full PE rate for N>=256). Tokens are processed in
two causal passes (halves of T) to fit SBUF.
"""

import functools

import numpy as np

B, T, C, H, D = 2, 2048, 1280, 10, 128
EPS = 1e-5
NHL = 3  # head slots per core (padded)
N_CORES = 8
NHALF = 2  # causal passes over T
# per-batch head groups (4th group padded with zero heads)
GROUPS = [[0, 1, 2], [3, 4, 5], [6, 7, 8], [9]]


def _emit(nc, tile, mybir, T, C, D, NHL, eps):
    F32 = mybir.dt.float32
    F32R = mybir.dt.float32r
    ActF = mybir.ActivationFunctionType
    CCH = C // 128  # contraction chunks
    TBN = T // 128  # 128-token blocks
    T2 = T // NHALF  # tokens per pass
    TB2 = T2 // 128
    Q42 = T2 // 512  # q supertiles per pass
    HD = NHL * D
    couts = []
    off = 0
    while off < C:
        w = min(512, C - off)
        couts.append((off, w))
        off += w

    xt = nc.dram_tensor("xt", [C, T], F32R, kind="ExternalInput")
    wqt = nc.dram_tensor("wqt", [C, HD], F32R, kind="ExternalInput")
    wkt = nc.dram_tensor("wkt", [C, HD], F32R, kind="ExternalInput")
    wvt = nc.dram_tensor("wvt", [C, HD], F32R, kind="ExternalInput")
    wpt = nc.dram_tensor("wpt", [HD, C], F32R, kind="ExternalInput")
    cs = nc.dram_tensor("cs", [D, T], F32, kind="ExternalInput")
    sc = nc.dram_tensor("sc", [D, T], F32, kind="ExternalInput")
    out = nc.dram_tensor("out", [T, C], F32, kind="ExternalOutput")

    from contextlib import ExitStack

    with ExitStack() as ctx:
        ctx.enter_context(nc.allow_low_precision(reason="fp32r matmul operands"))
        tc = ctx.enter_context(tile.TileContext(nc))
        pool = lambda n, b, **kw: ctx.enter_context(tc.tile_pool(name=n, bufs=b, **kw))
        per = pool("persist", 1)
        wvp = pool("wv", 1)
        wqkp = pool("wqk", 1)
        wptp = pool("wpt", 1)
        xtp = pool("xt", 1)
        qtp = pool("qt", 2)
        ytp = pool("yt", 1)
        tmp = pool("tmp", 2)
        sqp = pool("sqp", 2)
        ptp = pool("ptp", 3)
        rows = pool("rows", 1)
        oev = pool("oev", 2)
        psmm = pool("psmm", 3, space="PSUM")
        psacc = pool("psacc", 2, space="PSUM")
        psrow = pool("psrow", 2, space="PSUM")
        psq = pool("psq", 1, space="PSUM")
        if True:
            # ---- constants ----
            cs_t = per.tile([D, T], F32, tag="cs")
            sc_t = per.tile([D, T], F32, tag="sc")
            nc.sync.dma_start(cs_t[:], cs[:])
            nc.sync.dma_start(sc_t[:], sc[:])
            ones_f = per.tile([128, 128], F32, tag="onf")
            nc.vector.memset(ones_f[:], 1.0)
            zeros_f = per.tile([128, 384], F32, tag="zf")
            nc.vector.memset(zeros_f[:], 0.0)
            ones_col = per.tile([128, 1], F32R, tag="onc")
            nc.scalar.copy(ones_col[:], ones_f[:, 0:1])
            ones_row = per.tile([1, 128], F32R, tag="onr")
            nc.scalar.copy(ones_row[:], ones_f[0:1, :])
            # 0/1 mask: keep tq >= tk in [tk, tq] layout (upper incl diag)
            beps_row = per.tile([1, 1], F32, tag="bepsr")
            nc.vector.memset(beps_row[:], float(D * eps))
            beps_col = per.tile([128, 1], F32, tag="bepsc")
            nc.vector.memset(beps_col[:], float(eps))
            tri01 = per.tile([128, 128], F32, tag="tri")
            nc.vector.memset(tri01[:], 1.0)
            nc.gpsimd.affine_select(
                out=tri01[:],
                in_=tri01[:],
                compare_op=mybir.AluOpType.is_ge,
                fill=0.0,
                base=0,
                pattern=[[1, 128]],
                channel_multiplier=-1,
            )
            # rope half-mix selectors, M padded to 128 with disjoint columns:
            # y[0:64] = MA.T@t1 (cols 64-127 of MA zero),
            # y[64:128] = MB.T@t2 (cols 0-63 of MB zero); accumulated in PSUM.
            ma_f = per.tile([128, 128], F32, tag="maf")
            mb_f = per.tile([128, 128], F32, tag="mbf")
            nc.vector.memset(ma_f[:], 0.0)
            nc.vector.memset(mb_f[:], 0.0)
            # MA cols 0-63: +1 at k==m and k==m+64
            nc.gpsimd.affine_select(
                out=ma_f[:, 0:64], in_=ma_f[:, 0:64],
                compare_op=mybir.AluOpType.not_equal,
                fill=1.0, base=0,
                pattern=[[-1, 64]], channel_multiplier=1,
            )
            nc.gpsimd.affine_select(
                out=ma_f[:, 0:64], in_=ma_f[:, 0:64],
                compare_op=mybir.AluOpType.not_equal,
                fill=1.0, base=-64,
                pattern=[[-1, 64]], channel_multiplier=1,
            )
            # MB cols 64-127 (local m'): -1 at k==m', +1 at k==m'+64
            nc.gpsimd.affine_select(
                out=mb_f[:, 64:128], in_=mb_f[:, 64:128],
                compare_op=mybir.AluOpType.not_equal,
                fill=-1.0, base=0,
                pattern=[[-1, 64]], channel_multiplier=1,
            )
            nc.gpsimd.affine_select(
                out=mb_f[:, 64:128], in_=mb_f[:, 64:128],
                compare_op=mybir.AluOpType.not_equal,
                fill=1.0, base=-64,
                pattern=[[-1, 64]], channel_multiplier=1,
            )
            ma = per.tile([128, 128], F32R, tag="ma")
            mb = per.tile([128, 128], F32R, tag="mb")
            nc.scalar.copy(ma[:], ma_f[:])
            nc.scalar.copy(mb[:], mb_f[:])
            ones_col2 = per.tile([128, 2], F32R, tag="onc2")
            nc.scalar.copy(ones_col2[:], ones_f[:, 0:2])
            # V for all heads/all tokens: [tk-part, tb, h, d]
            v_t = per.tile([128, TBN, NHL, D], F32R, tag="v")
            # K.T per head, all tokens
            ktr = [per.tile([128, T], F32R, tag=f"ktr{h}", name=f"ktr{h}") for h in range(NHL)]
            rk_cols = [per.tile([128, TBN], F32, tag=f"rkc{h}", name=f"rkc{h}") for h in range(NHL)]

            # projection weights for V (resident; Q/K streamed per head/pass)
            wv = []
            for c in range(CCH):
                t = wvp.tile([128, HD], F32R, tag=f"wv{c}")
                nc.sync.dma_start(t[:], wvt[c * 128 : (c + 1) * 128, :])
                wv.append(t)
            # output-projection weights (resident across both passes)
            wp = {}
            for hh in range(NHL):
                for ci, (co, cw) in enumerate(couts):
                    t = wptp.tile([128, cw], F32R, tag=f"wp{hh}_{ci}")
                    nc.sync.dma_start(
                        t[:], wpt[hh * 128 : (hh + 1) * 128, co : co + cw]
                    )
                    wp[(hh, ci)] = t

            for hf in range(NHALF):
                toff = hf * T2
                # ---- load x.T chunks for this pass ----
                xc = []
                for c in range(CCH):
                    t = xtp.tile([128, T2], F32R, tag=f"x{c}")
                    nc.sync.dma_start(
                        t[:], xt[c * 128 : (c + 1) * 128, toff : toff + T2]
                    )
                    xc.append(t)

                # ---- V projection for this pass, all heads batched ----
                for tb in range(TB2):
                    gtb = hf * TB2 + tb
                    vp = psmm.tile([128, HD], F32, tag="mm")
                    for c in range(CCH):
                        nc.tensor.matmul(
                            vp[:],
                            xc[c][:, tb * 128 : (tb + 1) * 128],
                            wv[c][:],
                            start=(c == 0),
                            stop=(c == CCH - 1),
                        )
                    nc.scalar.copy(v_t[:, gtb, :, :], vp[:])

                # Y.T for this pass (all heads)
                ytn = ytp.tile([128, NHL, T2], F32R, tag="ytn")

                # ---- per-head: QK projection + rope + norm + attention ----
                for h in range(NHL):
                    wq = []
                    wk = []
                    for c in range(CCH):
                        tq = wqkp.tile([128, D], F32R, tag=f"wq{c}")
                        nc.sync.dma_start(
                            tq[:], wqt[c * 128 : (c + 1) * 128, h * D : (h + 1) * D]
                        )
                        wq.append(tq)
                        tk = wqkp.tile([128, D], F32R, tag=f"wk{c}")
                        nc.sync.dma_start(
                            tk[:], wkt[c * 128 : (c + 1) * 128, h * D : (h + 1) * D]
                        )
                        wk.append(tk)

                    qtn = qtp.tile([128, T2], F32R, tag="qtn")

                    for isq, (wt, dst, doff) in enumerate(
                        ((wq, qtn, 0), (wk, ktr[h], toff))
                    ):
                        qps = [
                            psmm.tile([128, 512], F32, tag="mm", name=f"qp{q4}")
                            for q4 in range(Q42)
                        ]
                        for c in range(CCH):
                            for q4 in range(Q42):
                                nc.tensor.matmul(
                                    qps[q4][:],
                                    wt[c][:],
                                    xc[c][:, q4 * 512 : (q4 + 1) * 512],
                                    start=(c == 0),
                                    stop=(c == CCH - 1),
                                )
                        for q4 in range(Q42):
                            gsl = slice(toff + q4 * 512, toff + (q4 + 1) * 512)
                            dsl = slice(doff + q4 * 512, doff + (q4 + 1) * 512)
                            qp = qps[q4]
                            t1 = tmp.tile([128, 512], F32R, tag="t1")
                            t2 = tmp.tile([128, 512], F32R, tag="t2")
                            nc.vector.tensor_mul(t1[:], qp[:], cs_t[:, gsl])
                            nc.vector.tensor_mul(t2[:], qp[:], sc_t[:, gsl])
                            rp = psmm.tile([128, 512], F32, tag="mm", name="rp")
                            nc.tensor.matmul(
                                rp[:], ma[:], t1[:], start=True, stop=False
                            )
                            nc.tensor.matmul(
                                rp[:], mb[:], t2[:], start=False, stop=True
                            )
                            nc.scalar.copy(dst[:, dsl], rp[:])
                        if isq == 0:
                            # q: rq = sqrt(1/ssq) (folds 1/sqrt(D); no eps --
                            # pad heads get nonzero Wq host-side), applied to
                            # qtn columns via ones-outer broadcast
                            for q4 in range(Q42):
                                lsl = slice(q4 * 512, (q4 + 1) * 512)
                                sq = sqp.tile([128, 512], F32R, tag="sq")
                                nc.vector.tensor_mul(
                                    sq[:], qtn[:, lsl].bitcast(F32),
                                    qtn[:, lsl].bitcast(F32),
                                )
                                ssq = psq.tile([1, 512], F32, tag="ssq")
                                nc.tensor.matmul(
                                    ssq[:], ones_col[:], sq[:], start=True, stop=True
                                )
                                rw = rows.tile([1, 512], F32, tag="rw")
                                nc.scalar.activation(rw[:], ssq[:], ActF.Ln)
                                rwr = rows.tile([1, 512], F32R, tag="rwr")
                                nc.scalar.activation(
                                    rwr[:], rw[:], ActF.Exp, scale=-0.5
                                )
                                bq = psmm.tile([128, 512], F32, tag="mm", name="bq")
                                nc.tensor.matmul(
                                    bq[:], ones_row[:], rwr[:], start=True, stop=True
                                )
                                nc.vector.tensor_mul(
                                    qtn[:, lsl], qtn[:, lsl].bitcast(F32), bq[:]
                                )
                        else:
                            # k: column form rk = 1/sqrt(ssq/D + eps) per tk
                            for tb in range(TB2):
                                gtb = hf * TB2 + tb
                                ksl = slice(toff + tb * 128, toff + (tb + 1) * 128)
                                sk = sqp.tile([128, 128], F32R, tag="sk")
                                nc.vector.tensor_mul(
                                    sk[:], ktr[h][:, ksl].bitcast(F32),
                                    ktr[h][:, ksl].bitcast(F32),
                                )
                                skp = psq.tile([128, 2], F32, tag="ssq", name="skp")
                                nc.tensor.matmul(
                                    skp[:], sk[:], ones_col2[:], start=True, stop=True
                                )
                                lk = rows.tile([128, 1], F32, tag="lk")
                                nc.scalar.activation(
                                    lk[:],
                                    skp[:, 0:1],
                                    ActF.Ln,
                                    scale=1.0 / D,
                                    bias=beps_col[:],
                                )
                                nc.scalar.activation(
                                    rk_cols[h][:, gtb : gtb + 1],
                                    lk[:],
                                    ActF.Exp,
                                    scale=-0.5,
                                )

                    # ---- attention for head h: kb-outer over both local
                    # q supertiles (reuses K/V stationary weights) ----
                    gq4s = [hf * Q42 + q4 for q4 in range(Q42)]
                    yts = [
                        psacc.tile([128, 512], F32, tag="acc", name=f"yt{q4}")
                        for q4 in range(Q42)
                    ]
                    csums = [
                        psrow.tile([1, 512], F32, tag="row", name=f"cs{q4}")
                        for q4 in range(Q42)
                    ]
                    kbmax = 4 * (gq4s[-1] + 1)
                    for kb in range(kbmax):
                        for q4 in range(Q42):
                            gq4 = gq4s[q4]
                            last_kb = 4 * gq4 + 3
                            if kb > last_kb:
                                continue
                            lsl = slice(q4 * 512, (q4 + 1) * 512)
                            st = psmm.tile([128, 512], F32, tag="mm", name="st")
                            nc.tensor.matmul(
                                st[:],
                                ktr[h][:, kb * 128 : (kb + 1) * 128],
                                qtn[:, lsl],
                                start=True,
                                stop=True,
                            )
                            pt = ptp.tile([128, 512], F32R, tag="pt")
                            j = kb - 4 * gq4
                            if j < 0:
                                nc.scalar.activation(
                                    pt[:],
                                    st[:],
                                    ActF.Exp,
                                    scale=rk_cols[h][:, kb : kb + 1],
                                )
                            else:
                                if j > 0:
                                    nc.scalar.copy(
                                        pt[:, : j * 128], zeros_f[:, : j * 128]
                                    )
                                nc.scalar.activation(
                                    pt[:, j * 128 :],
                                    st[:, j * 128 :],
                                    ActF.Exp,
                                    scale=rk_cols[h][:, kb : kb + 1],
                                )
                                dg = slice(j * 128, (j + 1) * 128)
                                nc.vector.tensor_mul(
                                    pt[:, dg], pt[:, dg].bitcast(F32), tri01[:]
                                )
                            nc.tensor.matmul(
                                yts[q4][:],
                                v_t[:, kb, h, :],
                                pt[:],
                                start=(kb == 0),
                                stop=(kb == last_kb),
                            )
                            nc.tensor.matmul(
                                csums[q4][:],
                                ones_col[:],
                                pt[:],
                                start=(kb == 0),
                                stop=(kb == last_kb),
                            )
                    for q4 in range(Q42):
                        lsl = slice(q4 * 512, (q4 + 1) * 512)
                        csr = rows.tile([1, 512], F32R, tag="csr")
                        nc.scalar.copy(csr[:], csums[q4][:])
                        bc = psmm.tile([128, 512], F32, tag="mm", name="bc")
                        nc.tensor.matmul(
                            bc[:], ones_row[:], csr[:], start=True, stop=True
                        )
                        bcs = tmp.tile([128, 512], F32, tag="bcs")
                        nc.vector.reciprocal_approx_fast(bcs[:], bc[:])
                        nc.vector.tensor_mul(ytn[:, h, lsl], yts[q4][:], bcs[:])

                # ---- output projection for this pass ----
                for tb in range(TB2):
                    for ci, (co, cw) in enumerate(couts):
                        op = psacc.tile([128, cw], F32, tag="acc")
                        for hh in range(NHL):
                            nc.tensor.matmul(
                                op[:],
                                ytn[:, hh, tb * 128 : (tb + 1) * 128],
                                wp[(hh, ci)][:],
                                start=(hh == 0),
                                stop=(hh == NHL - 1),
                            )
                        ot = oev.tile([128, cw], F32, tag="ot")
                        nc.vector.tensor_copy(ot[:], op[:])
                        nc.sync.dma_start(
                            out[toff + tb * 128 : toff + (tb + 1) * 128, co : co + cw],
                            ot[:],
                        )
    return nc


@functools.lru_cache(maxsize=4)
def _build(T_=T, C_=C, D_=D, NHL_=NHL, eps=EPS):
    import concourse.bacc as bacc
    import concourse.tile as tile
    from concourse import mybir

    nc = bacc.Bacc("TRN2", target_bir_lowering=False)
    _emit(nc, tile, mybir, T_, C_, D_, NHL_, eps)
    nc.compile()
    return nc


def _shard(x, cos, sin, Wq, Wk, Wv, Wproj):
    """Build the 8 per-core input maps."""
    HD = NHL * D
    cosT = np.ascontiguousarray(cos[0, 0].T.astype(np.float32))  # [64, T]
    sinT = np.ascontiguousarray(sin[0, 0].T.astype(np.float32))
    cs = np.concatenate([cosT, sinT], axis=0)  # [128, T]
    sc = np.concatenate([sinT, cosT], axis=0)

    def head_rows(W, heads, pad=0.0):
        rows = np.full((HD, C), pad, np.float32)
        for i, h in enumerate(heads):
            rows[i * D : (i + 1) * D] = W[h * D : (h + 1) * D]
        return rows

    in_maps = []
    for b in range(B):
        xtb = np.ascontiguousarray(x[b].T.astype(np.float32))  # [C, T]
        for heads in GROUPS:
            wq = np.ascontiguousarray(head_rows(Wq, heads, pad=0.01).T)  # [C, HD]
            wk = np.ascontiguousarray(head_rows(Wk, heads).T)
            wv = np.ascontiguousarray(head_rows(Wv, heads).T)
            # Wproj columns for these heads, transposed: [HD, C]
            wp = np.zeros((HD, C), np.float32)
            for i, h in enumerate(heads):
                wp[i * D : (i + 1) * D] = Wproj[:, h * D : (h + 1) * D].T
            in_maps.append(
                {"xt": xtb, "wqt": wq, "wkt": wk, "wvt": wv, "wpt": wp,
                 "cs": cs, "sc": sc}
            )
    return in_maps


def _gather(results):
    y = np.zeros((B, T, C), np.float32)
    for b in range(B):
        for g in range(len(GROUPS)):
            y[b] += results[b * len(GROUPS) + g]["out"]
    return y


def _run(in_maps, trace=False):
    from concourse.bass_utils import run_bass_kernel_spmd

    nc = _build()
    return run_bass_kernel_spmd(
        nc, in_maps, core_ids=list(range(N_CORES)), trace=trace
    )


def kernel(x, cos, sin, Wq, Wk, Wv, Wproj):
    ins = _shard(
        np.asarray(x), np.asarray(cos), np.asarray(sin),
        np.asarray(Wq), np.asarray(Wk), np.asarray(Wv), np.asarray(Wproj),
    )
    res = _run(ins, trace=False)
    return _gather(res.results)


def run_traced(x, cos, sin, Wq, Wk, Wv, Wproj):
    ins = _shard(
        np.asarray(x), np.asarray(cos), np.asarray(sin),
        np.asarray(Wq), np.asarray(Wk), np.asarray(Wv), np.asarray(Wproj),
    )
    res = _run(ins, trace=True)
    return _gather(res.results), res
